# revision 4
# baseline (speedup 1.0000x reference)
"""Trainium2 Bass kernel for the FilterAugment + 4-layer mirror-conv CNN.

Sharding: 8 cores = 4 samples x 2 H-halves. Odd cores work on an H-flipped
local frame so one SPMD program serves all cores (sample edge always at local
row 0); the host flips inputs/weights and un-flips the gathered output.

Per core:
  phase A: 4 separable gaussian blurs as dense banded-matrix matmuls
           (host-precomputed G matrices bake reflection padding), producing
           xpad [5, 266, 516] f16 in DRAM (channel 0 = nx copy), W-padded.
  phase B: layer0 conv as one K=125 matmul per row over a 25-offset
           "stacked" im2col of xpad, PReLU -> y0 f16.
  phase C: layers 1-3: per output row, y-part = 10 K=128 dy-pair matmuls +
           5 K=64 singles (dup'd input tile), x-part = 1 K=125 matmul;
           split across two PE column groups via tile_position for 2x
           concurrency. PReLU epilogue on ACT, combine on DVE.
All matmuls fp16 with fp32 PSUM accumulation.
"""

import os
import numpy as np
from itertools import product

B, H, W = 4, 512, 512
HALF = 256
PITCH = 516
SIGMAS = [4, 12, 48, 92]
KLENS = [s * 4 + 1 for s in SIGMAS]           # 17, 49, 193, 369
PADS = [(k - 1) // 2 for k in KLENS]          # 8, 24, 96, 184

_CACHE = {}


def _reflect(j, n=512):
    j = np.asarray(j)
    j = np.abs(j)
    return np.where(j > n - 1, 2 * (n - 1) - j, j)


def _build_g_matrices(gks, nrows):
    """Gf[i]: [512, 512] W-pass matrix; Gw_even/odd[i]: [512, nrows+4+2... ]
    columns of Gw = local x rows -2 .. nrows+7+... see REGX below."""
    regx = nrows + 10          # x rows [-2, nrows+8): count nrows+10
    gfs, gws_even, gws_odd = [], [], []
    for g in gks:
        g = np.asarray(g, np.float64)
        k = len(g)
        p = (k - 1) // 2
        t = np.arange(k)
        # W-pass: Gf[wi, wo] += g[t] where wi = reflect(wo + t - p)
        gf = np.zeros((512, 512), np.float64)
        for wo in range(512):
            wi = _reflect(wo + t - p)
            np.add.at(gf, (wi, wo), g)
        gfs.append(gf.astype(np.float16))
        # H-pass in local frame. column j <-> local x row r=j-2 (j<2: dup of row 2-j)
        gwe = np.zeros((512, regx), np.float64)
        gwo = np.zeros((512, regx), np.float64)
        for j in range(regx):
            r = j - 2
            if r < 0:
                r = -r
            he = _reflect(r + t - p)
            np.add.at(gwe, (he, j), g)
            ho = 511 - _reflect((511 - r) + t - p)
            np.add.at(gwo, (ho, j), g)
        gws_even.append(gwe.astype(np.float16))
        gws_odd.append(gwo.astype(np.float16))
    return gfs, gws_even, gws_odd


def _pack_weights(w, flip):
    """w: [64, cin, 5, 5] f32. Returns (wy [128, 960] or None, wx [125, 64])
    in f16. flip: reverse dy axis (odd cores)."""
    w = np.asarray(w, np.float32)
    if flip:
        w = w[:, :, ::-1, :]
    cin = w.shape[1]
    if cin == 5:
        wyo = None
        wxsrc = w
    else:
        wy = np.zeros((128, 960), np.float32)
        for pidx in range(2):
            for dx in range(5):
                blk = (pidx * 5 + dx) * 64
                for j in range(2):
                    # rows j*64+c = w[cout, c, 2p+j, dx]
                    wy[j * 64:(j + 1) * 64, blk:blk + 64] = \
                        w[:, 0:64, 2 * pidx + j, dx].T
        for dx in range(5):
            blk = (10 + dx) * 64
            wy[0:64, blk:blk + 64] = w[:, 0:64, 4, dx].T
        wyo = wy.astype(np.float16)
        wxsrc = w[:, 64:69, :, :]
    wx = np.zeros((125, 64), np.float32)
    for dy in range(5):
        for dx in range(5):
            o = dy * 5 + dx
            wx[o * 5:(o + 1) * 5, :] = wxsrc[:, :, dy, dx].T
    return wyo, wx.astype(np.float16)


def _chunks(total, r):
    c0 = 0
    while c0 < total:
        yield c0, min(r, total - c0)
        c0 += r


def _build_program(nrows, rchunk):
    import concourse.tile as tile
    from concourse import bacc, mybir

    F16 = mybir.dt.float16
    F32 = mybir.dt.float32
    PRELU = mybir.ActivationFunctionType.Prelu

    REGX = nrows + 10                    # xpad rows: local x rows [-2, nrows+8)
    REGY = [nrows + 6, nrows + 4, nrows + 2, nrows]   # y0..y2, out
    # pass1 k-tile lists per blur (band limited)
    KT1 = [[kt for kt in range(4) if kt * 128 < (nrows + 8) + p]
           for p in PADS]

    nc = bacc.Bacc("TRN2", target_bir_lowering=False, debug=False, num_devices=8)

    nx_e = nc.dram_tensor("nx16", [512, 512], F16, kind="ExternalInput")
    gw_e = [nc.dram_tensor(f"gw{i}", [512, REGX], F16, kind="ExternalInput")
            for i in range(4)]
    gf_e = [nc.dram_tensor(f"gf{i}", [512, 512], F16, kind="ExternalInput")
            for i in range(4)]
    w0s_e = nc.dram_tensor("w0s", [125, 64], F16, kind="ExternalInput")
    wx_e = [None] + [nc.dram_tensor(f"wx{l}", [125, 64], F16, kind="ExternalInput")
                     for l in (1, 2, 3)]
    wy_e = [None] + [nc.dram_tensor(f"wy{l}", [128, 960], F16, kind="ExternalInput")
                     for l in (1, 2, 3)]
    b_e = [nc.dram_tensor(f"b{l}", [64], F32, kind="ExternalInput") for l in range(4)]
    a_e = [nc.dram_tensor(f"a{l}", [1], F32, kind="ExternalInput") for l in range(4)]
    out_e = nc.dram_tensor("out", [64, nrows, 512], F32, kind="ExternalOutput")

    xpad = nc.dram_tensor("xpad", [5, REGX, PITCH], F16)
    yb = [nc.dram_tensor(f"yb{l}", [64, REGY[l], 512], F16) for l in range(3)]

    COLPAD = ((0, 2), (1, 1), (514, 510), (515, 509))

    with tile.TileContext(nc) as tc:
        # ---------------- phase A: blurs -> xpad ----------------
        with tc.tile_pool(name="nxp", bufs=1) as nxp, \
             tc.tile_pool(name="gwp", bufs=8) as gwp, \
             tc.tile_pool(name="gfp", bufs=8) as gfp, \
             tc.tile_pool(name="otp", bufs=8) as otp, \
             tc.tile_pool(name="xep", bufs=4) as xep, \
             tc.tile_pool(name="ps1", bufs=4, space="PSUM") as ps1p, \
             tc.tile_pool(name="ps2", bufs=4, space="PSUM") as ps2p:
            nxt = []
            for kt in range(4):
                t = nxp.tile([128, 512], F16, tag=f"nx{kt}")
                nc.sync.dma_start(t[:], nx_e[kt * 128:(kt + 1) * 128, :])
                nxt.append(t)
            # channel 0 = nx copy (rows j=2..REGX-1 <- nx rows 0..REGX-3)
            nxrows = REGX - 2          # = nrows + 8
            for kt in range(4):
                lo = kt * 128
                cnt = min(128, nxrows - lo)
                if cnt <= 0:
                    break
                nc.sync.dma_start(xpad[0, lo + 2: lo + 2 + cnt, 2:514],
                                  nxt[kt][0:cnt, :])
                for d, s in COLPAD:
                    nc.sync.dma_start(xpad[0, lo + 2: lo + 2 + cnt, d:d + 1],
                                      nxt[kt][0:cnt, s:s + 1])
            for j, srow in ((0, 2), (1, 1)):
                nc.sync.dma_start(xpad[0, j, 2:514], nxt[0][srow:srow + 1, :])
                for d, s in COLPAD:
                    nc.sync.dma_start(xpad[0, j, d:d + 1],
                                      nxt[0][srow:srow + 1, s:s + 1])
            mo_list = []
            o0 = 0
            while o0 < REGX:
                mo_list.append((o0, min(128, REGX - o0)))
                o0 += 128
            for i in range(4):
                gwt = {}
                for kt in KT1[i]:
                    t = gwp.tile([128, REGX], F16, tag="gw")
                    nc.sync.dma_start(t[:], gw_e[i][kt * 128:(kt + 1) * 128, :])
                    gwt[kt] = t
                gft = []
                for kt in range(4):
                    t = gfp.tile([128, 512], F16, tag="gf")
                    nc.sync.dma_start(t[:], gf_e[i][kt * 128:(kt + 1) * 128, :])
                    gft.append(t)
                outT = []
                for m in range(4):
                    ps = ps1p.tile([128, REGX], F32, tag="p1")
                    kts = KT1[i]
                    for j, kt in enumerate(kts):
                        nc.tensor.matmul(ps[:], nxt[kt][:, m * 128:(m + 1) * 128],
                                         gwt[kt][:],
                                         start=(j == 0), stop=(j == len(kts) - 1))
                    ot = otp.tile([128, REGX], F16, tag="oT")
                    nc.scalar.copy(ot[:], ps[:])
                    outT.append(ot)
                for (o0, osz) in mo_list:
                    ps = ps2p.tile([128, 512], F32, tag="p2")
                    for kw in range(4):
                        nc.tensor.matmul(ps[0:osz, :], outT[kw][:, o0:o0 + osz],
                                         gft[kw][:],
                                         start=(kw == 0), stop=(kw == 3))
                    xt = xep.tile([128, 512], F16, tag="xe")
                    nc.scalar.copy(xt[0:osz, :], ps[0:osz, :])
                    nc.sync.dma_start(xpad[i + 1, o0:o0 + osz, 2:514], xt[0:osz, :])
                    for d, s in COLPAD:
                        nc.sync.dma_start(xpad[i + 1, o0:o0 + osz, d:d + 1],
                                          xt[0:osz, s:s + 1])

        # shared pools for conv phases
        with tc.tile_pool(name="wp", bufs=1) as wp, \
             tc.tile_pool(name="xsp", bufs=2) as xsp, \
             tc.tile_pool(name="yp", bufs=2) as yp, \
             tc.tile_pool(name="cp", bufs=3) as cp, \
             tc.tile_pool(name="rb", bufs=4) as rbp, \
             tc.tile_pool(name="psc", bufs=3, space="PSUM") as pscp:

            bt, at = [], []
            for l in range(4):
                t = wp.tile([64, 1], F32, tag=f"b{l}")
                nc.sync.dma_start(t[:], b_e[l].ap().rearrange("c -> c ()"))
                bt.append(t)
                t = wp.tile([64, 1], F32, tag=f"a{l}")
                nc.sync.dma_start(
                    t[:], a_e[l].ap().rearrange("c -> c ()").broadcast_to((64, 1)))
                at.append(t)

            def build_xst(c0, rc):
                xs = xsp.tile([125, rchunk * 512], F16, tag="xs")
                for dy, dx in product(range(5), range(5)):
                    o = dy * 5 + dx
                    nc.sync.dma_start(
                        xs[o * 5:(o + 1) * 5, 0:rc * 512],
                        xpad[0:5, c0 + dy: c0 + dy + rc, dx: dx + 512])
                return xs

            # ---------------- phase B: layer 0 -> y0 ----------------
            w0t = wp.tile([125, 64], F16, tag="w0s")
            nc.sync.dma_start(w0t[:], w0s_e[:, :])
            for c0, rc in _chunks(REGY[0], rchunk):
                xs = build_xst(c0, rc)
                for r0 in range(0, rc, 2):
                    nr = min(2, rc - r0)
                    ps = pscp.tile([128, 1024], F32, tag="ps")
                    for hh in range(nr):
                        nc.tensor.matmul(ps[0:64, 512 * hh:512 * (hh + 1)],
                                         w0t[:], xs[:, (r0 + hh) * 512:(r0 + hh + 1) * 512],
                                         start=True, stop=True,
                                         skip_group_check=True)
                    rb = rbp.tile([64, 1024], F16, tag="rb16")
                    nc.scalar.activation(rb[:, 0:512 * nr], ps[0:64, 0:512 * nr],
                                         PRELU, bias=bt[0][:, 0:1], scale=1.0,
                                         alpha=at[0][:, 0:1])
                    nc.sync.dma_start(yb[0][:, c0 + r0: c0 + r0 + nr, :],
                                      rb[:, 0:512 * nr].rearrange(
                                          "c (r w) -> c r w", w=512))

            # ---------------- phase C: layers 1-3 ----------------
            for l in (1, 2, 3):
                wyt = wp.tile([128, 960], F16, tag=f"wy{l}")
                nc.sync.dma_start(wyt[:], wy_e[l][:, :])
                wxt = wp.tile([125, 64], F16, tag=f"wx{l}")
                nc.sync.dma_start(wxt[:], wx_e[l][:, :])
                ybin = yb[l - 1]
                reg_in, reg_out = REGY[l - 1], REGY[l]
                dst = out_e if l == 3 else yb[l]
                for c0, rc in _chunks(reg_out, rchunk):
                    nrows_t = rc + 4
                    yt = yp.tile([128, (rchunk + 4) * PITCH], F16, tag="yt")
                    ytv = yt[:].rearrange("c (r p) -> c r p", p=PITCH)
                    # main load rows [c0-2, c0+rc+2) of ybin, reflected at 0
                    if c0 == 0:
                        nc.sync.dma_start(ytv[0:64, 2:nrows_t, 2:514],
                                          ybin[:, 0:rc + 2, :])
                        for j, srow in ((0, 2), (1, 1)):
                            nc.sync.dma_start(ytv[0:64, j, 2:514],
                                              ybin[:, srow, :])
                            for d, s in COLPAD:
                                nc.sync.dma_start(ytv[0:64, j, d:d + 1],
                                                  ybin[:, srow, s:s + 1])
                        for d, s in COLPAD:
                            nc.sync.dma_start(ytv[0:64, 2:nrows_t, d:d + 1],
                                              ybin[:, 0:rc + 2, s:s + 1])
                    else:
                        nc.sync.dma_start(ytv[0:64, 0:nrows_t, 2:514],
                                          ybin[:, c0 - 2:c0 + rc + 2, :])
                        for d, s in COLPAD:
                            nc.sync.dma_start(ytv[0:64, 0:nrows_t, d:d + 1],
                                              ybin[:, c0 - 2:c0 + rc + 2, s:s + 1])
                    # dup: upper partitions = lower shifted one row
                    nc.sync.dma_start(yt[64:128, 0:(nrows_t - 1) * PITCH],
                                      yt[0:64, PITCH:nrows_t * PITCH])
                    xs = build_xst(c0, rc)
                    for r0 in range(0, rc, 2):
                        nr = min(2, rc - r0)
                        ps = pscp.tile([128, 1024], F32, tag="ps")
                        for hh in range(nr):
                            rr = r0 + hh
                            ocs = slice(512 * hh, 512 * (hh + 1))
                            # colA: pair0 dx0-4 (K=128), singles dx0-2 (K=64)
                            mmsA = [(wyt[:, (0 * 5 + dx) * 64:(0 * 5 + dx) * 64 + 64],
                                     yt[:, (rr + 0) * PITCH + dx:(rr + 0) * PITCH + dx + 512])
                                    for dx in range(5)]
                            mmsA += [(wyt[0:64, (10 + dx) * 64:(10 + dx) * 64 + 64],
                                      yt[0:64, (rr + 4) * PITCH + dx:(rr + 4) * PITCH + dx + 512])
                                     for dx in range(3)]
                            # colB: pair1 dx0-4 (K=128), singles dx3-4, x
                            mmsB = [(wyt[:, (1 * 5 + dx) * 64:(1 * 5 + dx) * 64 + 64],
                                     yt[:, (rr + 2) * PITCH + dx:(rr + 2) * PITCH + dx + 512])
                                    for dx in range(5)]
                            mmsB += [(wyt[0:64, (10 + dx) * 64:(10 + dx) * 64 + 64],
                                      yt[0:64, (rr + 4) * PITCH + dx:(rr + 4) * PITCH + dx + 512])
                                     for dx in (3, 4)]
                            mmsB += [(wxt[:], xs[:, rr * 512:(rr + 1) * 512])]
                            for k in range(8):
                                la, ra = mmsA[k]
                                nc.tensor.matmul(ps[0:64, ocs], la, ra,
                                                 start=(k == 0), stop=(k == 7),
                                                 skip_group_check=True)
                                lb, rb_ = mmsB[k]
                                nc.tensor.matmul(ps[64:128, ocs], lb, rb_,
                                                 start=(k == 0), stop=(k == 7),
                                                 skip_group_check=True,
                                                 tile_position=(0, 64))
                        w1024 = 512 * nr
                        ct = cp.tile([64, 1024], F32, tag="ct")
                        nc.scalar.copy(ct[:, 0:w1024], ps[64:128, 0:w1024])
                        tt = cp.tile([64, 1024], F32, tag="tt")
                        nc.vector.tensor_add(tt[:, 0:w1024], ps[0:64, 0:w1024],
                                             ct[:, 0:w1024])
                        if l == 3:
                            rb = rbp.tile([64, 1024], F32, tag="rb32")
                        else:
                            rb = rbp.tile([64, 1024], F16, tag="rb16")
                        nc.scalar.activation(rb[:, 0:w1024], tt[:, 0:w1024],
                                             PRELU, bias=bt[l][:, 0:1], scale=1.0,
                                             alpha=at[l][:, 0:1])
                        nc.sync.dma_start(
                            dst[:, c0 + r0: c0 + r0 + nr, :],
                            rb[:, 0:w1024].rearrange("c (r w) -> c r w", w=512))
    nc.compile()
    return nc


def _get_exec(nrows, rchunk):
    key = (nrows, rchunk)
    if key in _CACHE:
        return _CACHE[key]
    import jax
    import concourse.mybir as mybir
    from jax.sharding import Mesh, PartitionSpec
    from jax.experimental.shard_map import shard_map
    from concourse import bass2jax
    from concourse.bass2jax import _bass_exec_p, install_neuronx_cc_hook

    nc = _build_program(nrows, rchunk)
    install_neuronx_cc_hook()

    part_name = nc.partition_id_tensor.name if nc.partition_id_tensor else None
    in_names, out_names, out_avals, zero_shapes = [], [], [], []
    for alloc in nc.m.functions[0].allocations:
        if not isinstance(alloc, mybir.MemoryLocationSet):
            continue
        name = alloc.memorylocations[0].name
        if alloc.kind == "ExternalInput":
            if name != part_name:
                in_names.append(name)
        elif alloc.kind == "ExternalOutput":
            shape = tuple(alloc.tensor_shape)
            dtype = mybir.dt.np(alloc.dtype)
            out_names.append(name)
            out_avals.append(jax.core.ShapedArray(shape, dtype))
            zero_shapes.append((shape, dtype))
    n_params = len(in_names)
    n_outs = len(out_names)
    all_names = in_names + out_names
    if part_name is not None:
        all_names = all_names + [part_name]

    def _body(*args):
        operands = list(args)
        if part_name is not None:
            operands.append(bass2jax.partition_id_tensor())
        outs = _bass_exec_p.bind(
            *operands,
            out_avals=tuple(out_avals),
            in_names=tuple(all_names),
            out_names=tuple(out_names),
            lowering_input_output_aliases=(),
            sim_require_finite=True,
            sim_require_nnan=True,
            nc=nc,
        )
        return tuple(outs)

    devices = jax.devices()[:8]
    mesh = Mesh(np.asarray(devices), ("core",))
    in_specs = (PartitionSpec("core"),) * (n_params + n_outs)
    out_specs = (PartitionSpec("core"),) * n_outs
    donate = tuple(range(n_params, n_params + n_outs))
    sharded = jax.jit(
        shard_map(_body, mesh=mesh, in_specs=in_specs, out_specs=out_specs,
                  check_rep=False),
        donate_argnums=donate, keep_unused=True)

    def run(in_maps):
        concat_in = [np.concatenate([np.asarray(m[name]) for m in in_maps], axis=0)
                     for name in in_names]
        concat_zeros = [np.zeros((8 * s[0], *s[1:]), d) for s, d in zero_shapes]
        out_arrs = sharded(*concat_in, *concat_zeros)
        return [
            {name: np.asarray(out_arrs[i]).reshape(8, *out_avals[i].shape)[c]
             for i, name in enumerate(out_names)}
            for c in range(8)
        ]

    _CACHE[key] = run
    return run


def _make_in_maps(inputs, nrows):
    nx = np.asarray(inputs["nx"], np.float32)        # [4, 1, 512, 512]
    gks = [np.asarray(inputs[f"gk{i}"], np.float32) for i in range(4)]
    gfs, gwe, gwo = _build_g_matrices(gks, nrows)
    packs_even = [_pack_weights(inputs[f"w{l}"], False) for l in range(4)]
    packs_odd = [_pack_weights(inputs[f"w{l}"], True) for l in range(4)]
    in_maps = []
    for c in range(8):
        s, half = c >> 1, c & 1
        img = nx[s, 0]
        if half:
            img = img[::-1, :]
        m = {"nx16": np.ascontiguousarray(img).astype(np.float16)}
        gw = gwo if half else gwe
        for i in range(4):
            m[f"gw{i}"] = gw[i]
            m[f"gf{i}"] = gfs[i]
        packs = packs_odd if half else packs_even
        m["w0s"] = packs[0][1]
        for l in (1, 2, 3):
            m[f"wy{l}"] = packs[l][0]
            m[f"wx{l}"] = packs[l][1]
        for l in range(4):
            m[f"b{l}"] = np.asarray(inputs[f"b{l}"], np.float32)
            m[f"a{l}"] = np.asarray(inputs[f"a{l}"], np.float32).reshape(1)
        in_maps.append(m)
    return in_maps


def kernel(**inputs) -> np.ndarray:
    nrows = int(os.environ.get("BK_NROWS", HALF))
    rchunk = int(os.environ.get("BK_RCHUNK", 32))
    run = _get_exec(nrows, rchunk)
    in_maps = _make_in_maps(inputs, nrows)
    results = run(in_maps)
    out = np.zeros((B, 64, H, W), np.float32)
    for c in range(8):
        s, half = c >> 1, c & 1
        o = results[c]["out"]                      # [64, nrows, 512]
        if half:
            out[s, :, H - nrows:H, :] = o[:, ::-1, :]
        else:
            out[s, :, 0:nrows, :] = o
    return out


# revision 10
# speedup vs baseline: 275.6165x; 275.6165x over previous
"""Trainium2 Bass kernel for the FilterAugment + 4-layer mirror-conv CNN.

Sharding: 8 cores = 4 samples x 2 H-halves. Odd cores work on an H-flipped
local frame so one SPMD program serves all cores (sample edge always at local
row 0); the host flips inputs/weights and un-flips the gathered output.

Per core:
  phase A: 4 separable gaussian blurs as dense banded-matrix matmuls
           (host-precomputed G matrices bake reflection padding), producing
           xpad [5, 266, 516] f16 in DRAM (channel 0 = nx copy), W-padded.
  phase B: layer0 conv as one K=125 matmul per row over a 25-offset
           "stacked" im2col of xpad, PReLU -> y0 f16.
  phase C: layers 1-3: per output row, y-part = 10 K=128 dy-pair matmuls +
           5 K=64 singles (dup'd input tile), x-part = 1 K=125 matmul;
           split across two PE column groups via tile_position for 2x
           concurrency. PReLU epilogue on ACT, combine on DVE.
All matmuls fp16 with fp32 PSUM accumulation.
"""

import os
import numpy as np
from itertools import product

B, H, W = 4, 512, 512
HALF = 256
PITCH = 516
SIGMAS = [4, 12, 48, 92]
KLENS = [s * 4 + 1 for s in SIGMAS]           # 17, 49, 193, 369
PADS = [(k - 1) // 2 for k in KLENS]          # 8, 24, 96, 184

_CACHE = {}


def _reflect(j, n=512):
    j = np.asarray(j)
    j = np.abs(j)
    return np.where(j > n - 1, 2 * (n - 1) - j, j)


def _build_g_matrices(gks, nrows):
    """Gf[i]: [512, 512] W-pass matrix; Gw_even/odd[i]: [512, nrows+4+2... ]
    columns of Gw = local x rows -2 .. nrows+7+... see REGX below."""
    regx = nrows + 10          # x rows [-2, nrows+8): count nrows+10
    gfs, gws_even, gws_odd = [], [], []
    for g in gks:
        g = np.asarray(g, np.float64)
        k = len(g)
        p = (k - 1) // 2
        t = np.arange(k)
        # W-pass: Gf[wi, wo] += g[t] where wi = reflect(wo + t - p)
        gf = np.zeros((512, 512), np.float64)
        for wo in range(512):
            wi = _reflect(wo + t - p)
            np.add.at(gf, (wi, wo), g)
        gfs.append(gf.astype(np.float16))
        # H-pass in local frame. column j <-> local x row r=j-2 (j<2: dup of row 2-j)
        gwe = np.zeros((512, regx), np.float64)
        gwo = np.zeros((512, regx), np.float64)
        for j in range(regx):
            r = j - 2
            if r < 0:
                r = -r
            he = _reflect(r + t - p)
            np.add.at(gwe, (he, j), g)
            ho = 511 - _reflect((511 - r) + t - p)
            np.add.at(gwo, (ho, j), g)
        gws_even.append(gwe.astype(np.float16))
        gws_odd.append(gwo.astype(np.float16))
    return gfs, gws_even, gws_odd


def _pack_weights(w, flip):
    """w: [64, cin, 5, 5] f32. Returns (wy [128, 960] or None, wx [125, 64])
    in f16. flip: reverse dy axis (odd cores)."""
    w = np.asarray(w, np.float32)
    if flip:
        w = w[:, :, ::-1, :]
    cin = w.shape[1]
    if cin == 5:
        wyo = None
        wxsrc = w
    else:
        wy = np.zeros((128, 960), np.float32)
        for pidx in range(2):
            for dx in range(5):
                blk = (pidx * 5 + dx) * 64
                for j in range(2):
                    # rows j*64+c = w[cout, c, 2p+j, dx]
                    wy[j * 64:(j + 1) * 64, blk:blk + 64] = \
                        w[:, 0:64, 2 * pidx + j, dx].T
        for dx in range(5):
            blk = (10 + dx) * 64
            wy[0:64, blk:blk + 64] = w[:, 0:64, 4, dx].T
        wyo = wy.astype(np.float16)
        wxsrc = w[:, 64:69, :, :]
    wx = np.zeros((125, 64), np.float32)
    for dy in range(5):
        for dx in range(5):
            o = dy * 5 + dx
            wx[o * 5:(o + 1) * 5, :] = wxsrc[:, :, dy, dx].T
    return wyo, wx.astype(np.float16)


def _chunks(total, r):
    c0 = 0
    while c0 < total:
        yield c0, min(r, total - c0)
        c0 += r


def _build_program(nrows, rchunk):
    import concourse.tile as tile
    from concourse import bacc, mybir

    F16 = mybir.dt.float16
    F32 = mybir.dt.float32
    PRELU = mybir.ActivationFunctionType.Prelu

    REGX = nrows + 10                    # xpad rows: local x rows [-2, nrows+8)
    REGY = [nrows + 6, nrows + 4, nrows + 2, nrows]   # y0..y2, out
    # pass1 k-tile lists per blur (band limited)
    KT1 = [[kt for kt in range(4) if kt * 128 < (nrows + 8) + p]
           for p in PADS]

    nc = bacc.Bacc("TRN2", target_bir_lowering=False, debug=False, num_devices=8)

    nx_e = nc.dram_tensor("nx16", [512, 512], F16, kind="ExternalInput")
    gw_e = [nc.dram_tensor(f"gw{i}", [512, REGX], F16, kind="ExternalInput")
            for i in range(4)]
    gf_e = [nc.dram_tensor(f"gf{i}", [512, 512], F16, kind="ExternalInput")
            for i in range(4)]
    w0s_e = nc.dram_tensor("w0s", [125, 64], F16, kind="ExternalInput")
    wx_e = [None] + [nc.dram_tensor(f"wx{l}", [125, 64], F16, kind="ExternalInput")
                     for l in (1, 2, 3)]
    wy_e = [None] + [nc.dram_tensor(f"wy{l}", [128, 960], F16, kind="ExternalInput")
                     for l in (1, 2, 3)]
    b_e = [nc.dram_tensor(f"b{l}", [64], F32, kind="ExternalInput") for l in range(4)]
    a_e = [nc.dram_tensor(f"a{l}", [1], F32, kind="ExternalInput") for l in range(4)]
    out_e = nc.dram_tensor("out", [64, nrows, 512], F32, kind="ExternalOutput")

    xpad = nc.dram_tensor("xpad", [5, REGX, PITCH], F16)
    yb = [nc.dram_tensor(f"yb{l}", [64, REGY[l], 512], F16) for l in range(3)]

    COLPAD = ((0, 2), (1, 1), (514, 510), (515, 509))

    with tile.TileContext(nc) as tc:
        # ---------------- phase A: blurs -> xpad ----------------
        with tc.tile_pool(name="nxp", bufs=1) as nxp, \
             tc.tile_pool(name="gwp", bufs=8) as gwp, \
             tc.tile_pool(name="gfp", bufs=8) as gfp, \
             tc.tile_pool(name="otp", bufs=8) as otp, \
             tc.tile_pool(name="xep", bufs=4) as xep, \
             tc.tile_pool(name="ps1", bufs=4, space="PSUM") as ps1p, \
             tc.tile_pool(name="ps2", bufs=4, space="PSUM") as ps2p:
            nxt = []
            for kt in range(4):
                t = nxp.tile([128, 512], F16, tag=f"nx{kt}")
                nc.sync.dma_start(t[:], nx_e[kt * 128:(kt + 1) * 128, :])
                nxt.append(t)
            # channel 0 = nx copy (rows j=2..REGX-1 <- nx rows 0..REGX-3)
            nxrows = REGX - 2          # = nrows + 8
            for kt in range(4):
                lo = kt * 128
                cnt = min(128, nxrows - lo)
                if cnt <= 0:
                    break
                nc.sync.dma_start(xpad[0, lo + 2: lo + 2 + cnt, 2:514],
                                  nxt[kt][0:cnt, :])
                for d, s in COLPAD:
                    nc.sync.dma_start(xpad[0, lo + 2: lo + 2 + cnt, d:d + 1],
                                      nxt[kt][0:cnt, s:s + 1])
            for j, srow in ((0, 2), (1, 1)):
                nc.sync.dma_start(xpad[0, j, 2:514], nxt[0][srow:srow + 1, :])
                for d, s in COLPAD:
                    nc.sync.dma_start(xpad[0, j, d:d + 1],
                                      nxt[0][srow:srow + 1, s:s + 1])
            mo_list = []
            o0 = 0
            while o0 < REGX:
                mo_list.append((o0, min(128, REGX - o0)))
                o0 += 128
            for i in range(4):
                gwt = {}
                for kt in KT1[i]:
                    t = gwp.tile([128, REGX], F16, tag="gw")
                    nc.sync.dma_start(t[:], gw_e[i][kt * 128:(kt + 1) * 128, :])
                    gwt[kt] = t
                gft = []
                for kt in range(4):
                    t = gfp.tile([128, 512], F16, tag="gf")
                    nc.sync.dma_start(t[:], gf_e[i][kt * 128:(kt + 1) * 128, :])
                    gft.append(t)
                outT = []
                for m in range(4):
                    ps = ps1p.tile([128, REGX], F32, tag="p1")
                    kts = KT1[i]
                    for j, kt in enumerate(kts):
                        nc.tensor.matmul(ps[:], nxt[kt][:, m * 128:(m + 1) * 128],
                                         gwt[kt][:],
                                         start=(j == 0), stop=(j == len(kts) - 1))
                    ot = otp.tile([128, REGX], F16, tag="oT")
                    nc.scalar.copy(ot[:], ps[:])
                    outT.append(ot)
                for (o0, osz) in mo_list:
                    ps = ps2p.tile([128, 512], F32, tag="p2")
                    for kw in range(4):
                        nc.tensor.matmul(ps[0:osz, :], outT[kw][:, o0:o0 + osz],
                                         gft[kw][:],
                                         start=(kw == 0), stop=(kw == 3))
                    xt = xep.tile([128, 512], F16, tag="xe")
                    nc.scalar.copy(xt[0:osz, :], ps[0:osz, :])
                    nc.sync.dma_start(xpad[i + 1, o0:o0 + osz, 2:514], xt[0:osz, :])
                    for d, s in COLPAD:
                        nc.sync.dma_start(xpad[i + 1, o0:o0 + osz, d:d + 1],
                                          xt[0:osz, s:s + 1])

        # shared pools for conv phases
        with tc.tile_pool(name="wp", bufs=1) as wp, \
             tc.tile_pool(name="xsp", bufs=2) as xsp, \
             tc.tile_pool(name="yp", bufs=2) as yp, \
             tc.tile_pool(name="cp", bufs=3) as cp, \
             tc.tile_pool(name="rb", bufs=4) as rbp, \
             tc.tile_pool(name="psc", bufs=3, space="PSUM") as pscp:

            bt, at = [], []
            for l in range(4):
                t = wp.tile([64, 1], F32, tag=f"b{l}")
                nc.sync.dma_start(t[:], b_e[l].ap().rearrange("c -> c ()"))
                bt.append(t)
                t = wp.tile([64, 1], F32, tag=f"a{l}")
                nc.sync.dma_start(
                    t[:], a_e[l].ap().rearrange("c -> c ()").broadcast_to((64, 1)))
                at.append(t)

            def build_xst(c0, rc):
                xs = xsp.tile([125, rchunk * 512], F16, tag="xs")
                for dy, dx in product(range(5), range(5)):
                    o = dy * 5 + dx
                    nc.sync.dma_start(
                        xs[o * 5:(o + 1) * 5, 0:rc * 512],
                        xpad[0:5, c0 + dy: c0 + dy + rc, dx: dx + 512])
                return xs

            # ---------------- phase B: layer 0 -> y0 ----------------
            w0t = wp.tile([125, 64], F16, tag="w0s")
            nc.sync.dma_start(w0t[:], w0s_e[:, :])
            for c0, rc in _chunks(REGY[0], rchunk):
                xs = build_xst(c0, rc)
                for r0 in range(0, rc, 2):
                    nr = min(2, rc - r0)
                    ps = pscp.tile([128, 1024], F32, tag="ps")
                    for hh in range(nr):
                        nc.tensor.matmul(ps[0:64, 512 * hh:512 * (hh + 1)],
                                         w0t[:], xs[:, (r0 + hh) * 512:(r0 + hh + 1) * 512],
                                         start=True, stop=True,
                                         skip_group_check=True)
                    rb = rbp.tile([64, 1024], F16, tag="rb16")
                    nc.scalar.activation(rb[:, 0:512 * nr], ps[0:64, 0:512 * nr],
                                         PRELU, bias=bt[0][:, 0:1], scale=1.0,
                                         alpha=at[0][:, 0:1])
                    nc.sync.dma_start(yb[0][:, c0 + r0: c0 + r0 + nr, :],
                                      rb[:, 0:512 * nr].rearrange(
                                          "c (r w) -> c r w", w=512))

            # ---------------- phase C: layers 1-3 ----------------
            for l in (1, 2, 3):
                wyt = wp.tile([128, 960], F16, tag=f"wy{l}")
                nc.sync.dma_start(wyt[:], wy_e[l][:, :])
                wxt = wp.tile([125, 64], F16, tag=f"wx{l}")
                nc.sync.dma_start(wxt[:], wx_e[l][:, :])
                ybin = yb[l - 1]
                reg_in, reg_out = REGY[l - 1], REGY[l]
                dst = out_e if l == 3 else yb[l]
                for c0, rc in _chunks(reg_out, rchunk):
                    nrows_t = rc + 4
                    yt = yp.tile([128, (rchunk + 4) * PITCH], F16, tag="yt")
                    ytv = yt[:].rearrange("c (r p) -> c r p", p=PITCH)
                    # main load rows [c0-2, c0+rc+2) of ybin, reflected at 0
                    if c0 == 0:
                        nc.sync.dma_start(ytv[0:64, 2:nrows_t, 2:514],
                                          ybin[:, 0:rc + 2, :])
                        for j, srow in ((0, 2), (1, 1)):
                            nc.sync.dma_start(ytv[0:64, j, 2:514],
                                              ybin[:, srow, :])
                            for d, s in COLPAD:
                                nc.sync.dma_start(ytv[0:64, j, d:d + 1],
                                                  ybin[:, srow, s:s + 1])
                        for d, s in COLPAD:
                            nc.sync.dma_start(ytv[0:64, 2:nrows_t, d:d + 1],
                                              ybin[:, 0:rc + 2, s:s + 1])
                    else:
                        nc.sync.dma_start(ytv[0:64, 0:nrows_t, 2:514],
                                          ybin[:, c0 - 2:c0 + rc + 2, :])
                        for d, s in COLPAD:
                            nc.sync.dma_start(ytv[0:64, 0:nrows_t, d:d + 1],
                                              ybin[:, c0 - 2:c0 + rc + 2, s:s + 1])
                    # dup: upper partitions = lower shifted one row
                    nc.sync.dma_start(yt[64:128, 0:(nrows_t - 1) * PITCH],
                                      yt[0:64, PITCH:nrows_t * PITCH])
                    xs = build_xst(c0, rc)
                    for r0 in range(0, rc, 2):
                        nr = min(2, rc - r0)
                        ps = pscp.tile([128, 1024], F32, tag="ps")
                        for hh in range(nr):
                            rr = r0 + hh
                            ocs = slice(512 * hh, 512 * (hh + 1))
                            # colA: pair0 dx0-4 (K=128), singles dx0-2 (K=64)
                            mmsA = [(wyt[:, (0 * 5 + dx) * 64:(0 * 5 + dx) * 64 + 64],
                                     yt[:, (rr + 0) * PITCH + dx:(rr + 0) * PITCH + dx + 512])
                                    for dx in range(5)]
                            mmsA += [(wyt[0:64, (10 + dx) * 64:(10 + dx) * 64 + 64],
                                      yt[0:64, (rr + 4) * PITCH + dx:(rr + 4) * PITCH + dx + 512])
                                     for dx in range(3)]
                            # colB: pair1 dx0-4 (K=128), singles dx3-4, x
                            mmsB = [(wyt[:, (1 * 5 + dx) * 64:(1 * 5 + dx) * 64 + 64],
                                     yt[:, (rr + 2) * PITCH + dx:(rr + 2) * PITCH + dx + 512])
                                    for dx in range(5)]
                            mmsB += [(wyt[0:64, (10 + dx) * 64:(10 + dx) * 64 + 64],
                                      yt[0:64, (rr + 4) * PITCH + dx:(rr + 4) * PITCH + dx + 512])
                                     for dx in (3, 4)]
                            mmsB += [(wxt[:], xs[:, rr * 512:(rr + 1) * 512])]
                            for k in range(8):
                                la, ra = mmsA[k]
                                nc.tensor.matmul(ps[0:64, ocs], la, ra,
                                                 start=(k == 0), stop=(k == 7),
                                                 skip_group_check=True)
                                lb, rb_ = mmsB[k]
                                nc.tensor.matmul(ps[64:128, ocs], lb, rb_,
                                                 start=(k == 0), stop=(k == 7),
                                                 skip_group_check=True,
                                                 tile_position=(0, 64))
                        w1024 = 512 * nr
                        ct = cp.tile([64, 1024], F32, tag="ct")
                        nc.scalar.copy(ct[:, 0:w1024], ps[64:128, 0:w1024])
                        tt = cp.tile([64, 1024], F32, tag="tt")
                        nc.vector.tensor_add(tt[:, 0:w1024], ps[0:64, 0:w1024],
                                             ct[:, 0:w1024])
                        if l == 3:
                            rb = rbp.tile([64, 1024], F32, tag="rb32")
                        else:
                            rb = rbp.tile([64, 1024], F16, tag="rb16")
                        nc.scalar.activation(rb[:, 0:w1024], tt[:, 0:w1024],
                                             PRELU, bias=bt[l][:, 0:1], scale=1.0,
                                             alpha=at[l][:, 0:1])
                        nc.sync.dma_start(
                            dst[:, c0 + r0: c0 + r0 + nr, :],
                            rb[:, 0:w1024].rearrange("c (r w) -> c r w", w=512))
    nc.compile()
    return nc


def _get_exec(nrows, rchunk):
    key = (nrows, rchunk)
    if key in _CACHE:
        return _CACHE[key]
    import jax
    import concourse.mybir as mybir
    from jax.sharding import Mesh, PartitionSpec
    from jax.experimental.shard_map import shard_map
    from concourse import bass2jax
    from concourse.bass2jax import _bass_exec_p, install_neuronx_cc_hook

    nc = _build_program(nrows, rchunk)
    install_neuronx_cc_hook()

    part_name = nc.partition_id_tensor.name if nc.partition_id_tensor else None
    in_names, out_names, out_avals, zero_shapes = [], [], [], []
    for alloc in nc.m.functions[0].allocations:
        if not isinstance(alloc, mybir.MemoryLocationSet):
            continue
        name = alloc.memorylocations[0].name
        if alloc.kind == "ExternalInput":
            if name != part_name:
                in_names.append(name)
        elif alloc.kind == "ExternalOutput":
            shape = tuple(alloc.tensor_shape)
            dtype = mybir.dt.np(alloc.dtype)
            out_names.append(name)
            out_avals.append(jax.core.ShapedArray(shape, dtype))
            zero_shapes.append((shape, dtype))
    n_params = len(in_names)
    n_outs = len(out_names)
    all_names = in_names + out_names
    if part_name is not None:
        all_names = all_names + [part_name]

    import jax.numpy as jnp

    def _call_once(ins, out_bufs):
        operands = list(ins) + list(out_bufs)
        if part_name is not None:
            operands.append(bass2jax.partition_id_tensor())
        outs = _bass_exec_p.bind(
            *operands,
            out_avals=tuple(out_avals),
            in_names=tuple(all_names),
            out_names=tuple(out_names),
            lowering_input_output_aliases=(),
            sim_require_finite=True,
            sim_require_nnan=True,
            nc=nc,
        )
        return tuple(outs)

    def _body_iters(iters):
        def f(*args):
            ins = args[:n_params]
            bufs = list(args[n_params:n_params + n_outs])
            for _ in range(iters):
                bufs = list(_call_once(ins, bufs))
            return tuple(bufs)
        return f

    _body = _body_iters(1)

    devices = jax.devices()[:8]
    mesh = Mesh(np.asarray(devices), ("core",))
    in_specs = (PartitionSpec("core"),) * (n_params + n_outs)
    out_specs = (PartitionSpec("core"),) * n_outs
    donate = tuple(range(n_params, n_params + n_outs))
    sharded = jax.jit(
        shard_map(_body, mesh=mesh, in_specs=in_specs, out_specs=out_specs,
                  check_rep=False),
        donate_argnums=donate, keep_unused=True)

    def _concat_in(in_maps):
        return [np.concatenate([np.asarray(m[name]) for m in in_maps], axis=0)
                for name in in_names]

    def _concat_zeros():
        return [np.zeros((8 * s[0], *s[1:]), d) for s, d in zero_shapes]

    def run(in_maps):
        out_arrs = sharded(*_concat_in(in_maps), *_concat_zeros())
        return [
            {name: np.asarray(out_arrs[i]).reshape(8, *out_avals[i].shape)[c]
             for i, name in enumerate(out_names)}
            for c in range(8)
        ]

    def time_exec(in_maps, repeats=10):
        """Min wall seconds of one execution with device-resident I/O."""
        import time as _time
        dev_in = [jax.device_put(x) for x in _concat_in(in_maps)]
        fn = jax.jit(
            shard_map(_body, mesh=mesh, in_specs=in_specs,
                      out_specs=out_specs, check_rep=False),
            keep_unused=True)
        zz = [jax.device_put(z) for z in _concat_zeros()]
        outs = fn(*dev_in, *zz)          # compile + warm
        jax.block_until_ready(outs)
        best = float("inf")
        for _ in range(repeats):
            t0 = _time.time()
            outs = fn(*dev_in, *zz)
            jax.block_until_ready(outs)
            best = min(best, _time.time() - t0)
        return best

    run.time_exec = time_exec
    _CACHE[key] = run
    return run


def baseline_time(repeats=10):
    """Time an (almost) empty program with the same output signature, to
    subtract dispatch/RPC overhead from time_exec."""
    if "baseline" in _CACHE:
        return _CACHE["baseline"](repeats)
    import jax
    import concourse.tile as tile
    from concourse import bacc, mybir

    F32 = mybir.dt.float32
    nc = bacc.Bacc("TRN2", target_bir_lowering=False, debug=False, num_devices=8)
    x_e = nc.dram_tensor("x", [64, 512], F32, kind="ExternalInput")
    out_e = nc.dram_tensor("out", [64, 256, 512], F32, kind="ExternalOutput")
    with tile.TileContext(nc) as tc:
        with tc.tile_pool(name="sb", bufs=1) as sb:
            t = sb.tile([64, 512], F32)
            nc.sync.dma_start(t[:], x_e[:, :])
            nc.sync.dma_start(out_e[:, 0, :], t[:])
    nc.compile()
    runner = _wrap_exec(nc)

    def bt(reps):
        import time as _time
        import numpy as _np
        in_maps = [{"x": _np.zeros((64, 512), _np.float32)} for _ in range(8)]
        return runner(in_maps, reps)

    _CACHE["baseline"] = bt
    return bt(repeats)


def _wrap_exec(nc):
    """Minimal timed executor for an arbitrary compiled nc (used by baseline)."""
    import jax
    import concourse.mybir as mybir
    from jax.sharding import Mesh, PartitionSpec
    from jax.experimental.shard_map import shard_map
    from concourse import bass2jax
    from concourse.bass2jax import _bass_exec_p, install_neuronx_cc_hook
    install_neuronx_cc_hook()

    part_name = nc.partition_id_tensor.name if nc.partition_id_tensor else None
    in_names, out_names, out_avals, zero_shapes = [], [], [], []
    for alloc in nc.m.functions[0].allocations:
        if not isinstance(alloc, mybir.MemoryLocationSet):
            continue
        name = alloc.memorylocations[0].name
        if alloc.kind == "ExternalInput":
            if name != part_name:
                in_names.append(name)
        elif alloc.kind == "ExternalOutput":
            shape = tuple(alloc.tensor_shape)
            dtype = mybir.dt.np(alloc.dtype)
            out_names.append(name)
            out_avals.append(jax.core.ShapedArray(shape, dtype))
            zero_shapes.append((shape, dtype))
    n_params, n_outs = len(in_names), len(out_names)
    all_names = in_names + out_names + ([part_name] if part_name else [])

    def _body(*args):
        operands = list(args)
        if part_name is not None:
            operands.append(bass2jax.partition_id_tensor())
        return tuple(_bass_exec_p.bind(
            *operands, out_avals=tuple(out_avals), in_names=tuple(all_names),
            out_names=tuple(out_names), lowering_input_output_aliases=(),
            sim_require_finite=True, sim_require_nnan=True, nc=nc))

    devices = jax.devices()[:8]
    mesh = Mesh(np.asarray(devices), ("core",))
    fn = jax.jit(
        shard_map(_body, mesh=mesh,
                  in_specs=(PartitionSpec("core"),) * (n_params + n_outs),
                  out_specs=(PartitionSpec("core"),) * n_outs,
                  check_rep=False),
        keep_unused=True)

    def timed(in_maps, repeats):
        import time as _time
        dev_in = [jax.device_put(
            np.concatenate([np.asarray(m[nm]) for m in in_maps], axis=0))
            for nm in in_names]
        zz = [jax.device_put(np.zeros((8 * s[0], *s[1:]), d))
              for s, d in zero_shapes]
        outs = fn(*dev_in, *zz)
        jax.block_until_ready(outs)
        best = float("inf")
        for _ in range(repeats):
            t0 = _time.time()
            outs = fn(*dev_in, *zz)
            jax.block_until_ready(outs)
            best = min(best, _time.time() - t0)
        return best

    return timed


def _make_in_maps(inputs, nrows):
    nx = np.asarray(inputs["nx"], np.float32)        # [4, 1, 512, 512]
    gks = [np.asarray(inputs[f"gk{i}"], np.float32) for i in range(4)]
    gfs, gwe, gwo = _build_g_matrices(gks, nrows)
    packs_even = [_pack_weights(inputs[f"w{l}"], False) for l in range(4)]
    packs_odd = [_pack_weights(inputs[f"w{l}"], True) for l in range(4)]
    in_maps = []
    for c in range(8):
        s, half = c >> 1, c & 1
        img = nx[s, 0]
        if half:
            img = img[::-1, :]
        m = {"nx16": np.ascontiguousarray(img).astype(np.float16)}
        gw = gwo if half else gwe
        for i in range(4):
            m[f"gw{i}"] = gw[i]
            m[f"gf{i}"] = gfs[i]
        packs = packs_odd if half else packs_even
        m["w0s"] = packs[0][1]
        for l in (1, 2, 3):
            m[f"wy{l}"] = packs[l][0]
            m[f"wx{l}"] = packs[l][1]
        for l in range(4):
            m[f"b{l}"] = np.asarray(inputs[f"b{l}"], np.float32)
            m[f"a{l}"] = np.asarray(inputs[f"a{l}"], np.float32).reshape(1)
        in_maps.append(m)
    return in_maps


def kernel(**inputs) -> np.ndarray:
    nrows = int(os.environ.get("BK_NROWS", HALF))
    rchunk = int(os.environ.get("BK_RCHUNK", 32))
    run = _get_exec(nrows, rchunk)
    in_maps = _make_in_maps(inputs, nrows)
    results = run(in_maps)
    out = np.zeros((B, 64, H, W), np.float32)
    for c in range(8):
        s, half = c >> 1, c & 1
        o = results[c]["out"]                      # [64, nrows, 512]
        if half:
            out[s, :, H - nrows:H, :] = o[:, ::-1, :]
        else:
            out[s, :, 0:nrows, :] = o
    return out


# revision 13
# speedup vs baseline: 936.5826x; 3.3981x over previous
"""Trainium2 Bass kernel for the FilterAugment + 4-layer mirror-conv CNN.

Sharding: 8 cores = 4 samples x 2 H-halves. Odd cores work on an H-flipped
local frame so one SPMD program serves all cores (sample edge always at local
row 0); the host flips inputs/weights and un-flips the gathered output.

Per core:
  phase A: 4 separable gaussian blurs as dense banded-matrix matmuls
           (host-precomputed G matrices bake reflection padding), producing
           xpad [5, 266, 516] f16 in DRAM (channel 0 = nx copy), W-padded.
  phase B: layer0 conv as one K=125 matmul per row over a 25-offset
           "stacked" im2col of xpad, PReLU -> y0 f16.
  phase C: layers 1-3: per output row, y-part = 10 K=128 dy-pair matmuls +
           5 K=64 singles (dup'd input tile), x-part = 1 K=125 matmul;
           split across two PE column groups via tile_position for 2x
           concurrency. PReLU epilogue on ACT, combine on DVE.
All matmuls fp16 with fp32 PSUM accumulation.
"""

import os
import numpy as np
from itertools import product

B, H, W = 4, 512, 512
HALF = 256
PITCH = 516
SIGMAS = [4, 12, 48, 92]
KLENS = [s * 4 + 1 for s in SIGMAS]           # 17, 49, 193, 369
PADS = [(k - 1) // 2 for k in KLENS]          # 8, 24, 96, 184

_CACHE = {}


def _reflect(j, n=512):
    j = np.asarray(j)
    j = np.abs(j)
    return np.where(j > n - 1, 2 * (n - 1) - j, j)


def _build_g_matrices(gks, nrows):
    """Gf[i]: [512, 512] W-pass matrix; Gw_even/odd[i]: [512, nrows+4+2... ]
    columns of Gw = local x rows -2 .. nrows+7+... see REGX below."""
    regx = nrows + 10          # x rows [-2, nrows+8): count nrows+10
    gfs, gws_even, gws_odd = [], [], []
    for g in gks:
        g = np.asarray(g, np.float64)
        k = len(g)
        p = (k - 1) // 2
        t = np.arange(k)
        # W-pass: Gf[wi, wo] += g[t] where wi = reflect(wo + t - p)
        gf = np.zeros((512, 512), np.float64)
        for wo in range(512):
            wi = _reflect(wo + t - p)
            np.add.at(gf, (wi, wo), g)
        gfs.append(gf.astype(np.float16))
        # H-pass in local frame. column j <-> local x row r=j-2 (j<2: dup of row 2-j)
        gwe = np.zeros((512, regx), np.float64)
        gwo = np.zeros((512, regx), np.float64)
        for j in range(regx):
            r = j - 2
            if r < 0:
                r = -r
            he = _reflect(r + t - p)
            np.add.at(gwe, (he, j), g)
            ho = 511 - _reflect((511 - r) + t - p)
            np.add.at(gwo, (ho, j), g)
        gws_even.append(gwe.astype(np.float16))
        gws_odd.append(gwo.astype(np.float16))
    return gfs, gws_even, gws_odd


def _pack_weights(w, flip):
    """w: [64, cin, 5, 5] f32. Returns (wy [128, 960] or None, wx [125, 64])
    in f16. flip: reverse dy axis (odd cores)."""
    w = np.asarray(w, np.float32)
    if flip:
        w = w[:, :, ::-1, :]
    cin = w.shape[1]
    if cin == 5:
        wyo = None
        wxsrc = w
    else:
        wy = np.zeros((128, 960), np.float32)
        for pidx in range(2):
            for dx in range(5):
                blk = (pidx * 5 + dx) * 64
                for j in range(2):
                    # rows j*64+c = w[cout, c, 2p+j, dx]
                    wy[j * 64:(j + 1) * 64, blk:blk + 64] = \
                        w[:, 0:64, 2 * pidx + j, dx].T
        for dx in range(5):
            blk = (10 + dx) * 64
            wy[0:64, blk:blk + 64] = w[:, 0:64, 4, dx].T
        wyo = wy.astype(np.float16)
        wxsrc = w[:, 64:69, :, :]
    wx = np.zeros((125, 64), np.float32)
    for dy in range(5):
        for dx in range(5):
            o = dy * 5 + dx
            wx[o * 5:(o + 1) * 5, :] = wxsrc[:, :, dy, dx].T
    return wyo, wx.astype(np.float16)


def _chunks(total, r):
    c0 = 0
    while c0 < total:
        yield c0, min(r, total - c0)
        c0 += r


def _build_program(nrows, rchunk):
    import concourse.tile as tile
    from concourse import bacc, mybir

    F16 = mybir.dt.float16
    F32 = mybir.dt.float32
    PRELU = mybir.ActivationFunctionType.Prelu

    REGX = nrows + 10                    # xpad rows: local x rows [-2, nrows+8)
    REGY = [nrows + 6, nrows + 4, nrows + 2, nrows]   # y0..y2, out
    # pass1 k-tile lists per blur (band limited)
    KT1 = [[kt for kt in range(4) if kt * 128 < (nrows + 8) + p]
           for p in PADS]

    nc = bacc.Bacc("TRN2", target_bir_lowering=False, debug=False, num_devices=8)

    nx_e = nc.dram_tensor("nx16", [512, 512], F16, kind="ExternalInput")
    gw_e = [nc.dram_tensor(f"gw{i}", [512, REGX], F16, kind="ExternalInput")
            for i in range(4)]
    gf_e = [nc.dram_tensor(f"gf{i}", [512, 512], F16, kind="ExternalInput")
            for i in range(4)]
    w0s_e = nc.dram_tensor("w0s", [125, 64], F16, kind="ExternalInput")
    wx_e = [None] + [nc.dram_tensor(f"wx{l}", [125, 64], F16, kind="ExternalInput")
                     for l in (1, 2, 3)]
    wy_e = [None] + [nc.dram_tensor(f"wy{l}", [128, 960], F16, kind="ExternalInput")
                     for l in (1, 2, 3)]
    b_e = [nc.dram_tensor(f"b{l}", [64], F32, kind="ExternalInput") for l in range(4)]
    a_e = [nc.dram_tensor(f"a{l}", [1], F32, kind="ExternalInput") for l in range(4)]
    out_e = nc.dram_tensor("out", [64, nrows, 512], F32, kind="ExternalOutput")

    xpad = nc.dram_tensor("xpad", [5, REGX, PITCH], F16)
    yb = [nc.dram_tensor(f"yb{l}", [64, REGY[l], 512], F16) for l in range(3)]

    skips = set(os.environ.get("BK_SKIP", "").split(","))
    COLPAD = () if "colpad" in skips else ((0, 2), (1, 1), (514, 510), (515, 509))

    with tile.TileContext(nc) as tc:
        # ---------------- phase A: blurs -> xpad ----------------
        with tc.tile_pool(name="nxp", bufs=1) as nxp, \
             tc.tile_pool(name="gwp", bufs=8) as gwp, \
             tc.tile_pool(name="gfp", bufs=8) as gfp, \
             tc.tile_pool(name="otp", bufs=8) as otp, \
             tc.tile_pool(name="xep", bufs=4) as xep, \
             tc.tile_pool(name="ps1", bufs=4, space="PSUM") as ps1p, \
             tc.tile_pool(name="ps2", bufs=4, space="PSUM") as ps2p:
            nxt = []
            for kt in range(4):
                t = nxp.tile([128, 512], F16, tag=f"nx{kt}")
                nc.sync.dma_start(t[:], nx_e[kt * 128:(kt + 1) * 128, :])
                nxt.append(t)
            # channel 0 = nx copy (rows j=2..REGX-1 <- nx rows 0..REGX-3)
            nxrows = REGX - 2          # = nrows + 8
            for kt in range(4):
                lo = kt * 128
                cnt = min(128, nxrows - lo)
                if cnt <= 0:
                    break
                nc.sync.dma_start(xpad[0, lo + 2: lo + 2 + cnt, 2:514],
                                  nxt[kt][0:cnt, :])
                for d, s in COLPAD:
                    nc.sync.dma_start(xpad[0, lo + 2: lo + 2 + cnt, d:d + 1],
                                      nxt[kt][0:cnt, s:s + 1])
            for j, srow in ((0, 2), (1, 1)):
                nc.sync.dma_start(xpad[0, j, 2:514], nxt[0][srow:srow + 1, :])
                for d, s in COLPAD:
                    nc.sync.dma_start(xpad[0, j, d:d + 1],
                                      nxt[0][srow:srow + 1, s:s + 1])
            mo_list = []
            o0 = 0
            while o0 < REGX:
                mo_list.append((o0, min(128, REGX - o0)))
                o0 += 128
            for i in range(4):
                gwt = {}
                for kt in KT1[i]:
                    t = gwp.tile([128, REGX], F16, tag="gw")
                    nc.sync.dma_start(t[:], gw_e[i][kt * 128:(kt + 1) * 128, :])
                    gwt[kt] = t
                gft = []
                for kt in range(4):
                    t = gfp.tile([128, 512], F16, tag="gf")
                    nc.sync.dma_start(t[:], gf_e[i][kt * 128:(kt + 1) * 128, :])
                    gft.append(t)
                outT = []
                for m in range(4):
                    ps = ps1p.tile([128, REGX], F32, tag="p1")
                    kts = KT1[i]
                    for j, kt in enumerate(kts):
                        nc.tensor.matmul(ps[:], nxt[kt][:, m * 128:(m + 1) * 128],
                                         gwt[kt][:],
                                         start=(j == 0), stop=(j == len(kts) - 1))
                    ot = otp.tile([128, REGX], F16, tag="oT")
                    nc.scalar.copy(ot[:], ps[:])
                    outT.append(ot)
                for (o0, osz) in mo_list:
                    ps = ps2p.tile([128, 512], F32, tag="p2")
                    for kw in range(4):
                        nc.tensor.matmul(ps[0:osz, :], outT[kw][:, o0:o0 + osz],
                                         gft[kw][:],
                                         start=(kw == 0), stop=(kw == 3))
                    xt = xep.tile([128, 512], F16, tag="xe")
                    nc.scalar.copy(xt[0:osz, :], ps[0:osz, :])
                    nc.sync.dma_start(xpad[i + 1, o0:o0 + osz, 2:514], xt[0:osz, :])
                    for d, s in COLPAD:
                        nc.sync.dma_start(xpad[i + 1, o0:o0 + osz, d:d + 1],
                                          xt[0:osz, s:s + 1])

        # shared pools for conv phases
        with tc.tile_pool(name="wp", bufs=1) as wp, \
             tc.tile_pool(name="xsp", bufs=2) as xsp, \
             tc.tile_pool(name="yp", bufs=2) as yp, \
             tc.tile_pool(name="cp", bufs=3) as cp, \
             tc.tile_pool(name="rb", bufs=4) as rbp, \
             tc.tile_pool(name="psc", bufs=3, space="PSUM") as pscp:

            bt, at = [], []
            for l in range(4):
                t = wp.tile([64, 1], F32, tag=f"b{l}")
                nc.sync.dma_start(t[:], b_e[l].ap().rearrange("c -> c ()"))
                bt.append(t)
                t = wp.tile([64, 1], F32, tag=f"a{l}")
                nc.sync.dma_start(
                    t[:], a_e[l].ap().rearrange("c -> c ()").broadcast_to((64, 1)))
                at.append(t)

            def build_xst(c0, rc):
                xs = xsp.tile([125, rchunk * 512], F16, tag="xs")
                if "xst" in skips:
                    nc.sync.dma_start(xs[0:5, 0:rc * 512],
                                      xpad[0:5, c0: c0 + rc, 0: 512])
                    return xs
                for dy, dx in product(range(5), range(5)):
                    o = dy * 5 + dx
                    nc.sync.dma_start(
                        xs[o * 5:(o + 1) * 5, 0:rc * 512],
                        xpad[0:5, c0 + dy: c0 + dy + rc, dx: dx + 512])
                return xs

            # ---------------- phase B: layer 0 -> y0 ----------------
            w0t = wp.tile([125, 64], F16, tag="w0s")
            nc.sync.dma_start(w0t[:], w0s_e[:, :])
            for c0, rc in _chunks(REGY[0], rchunk):
                xs = build_xst(c0, rc)
                for r0 in range(0, rc, 2):
                    nr = min(2, rc - r0)
                    ps = pscp.tile([128, 1024], F32, tag="ps")
                    for hh in range(nr):
                        nc.tensor.matmul(ps[0:64, 512 * hh:512 * (hh + 1)],
                                         w0t[:], xs[:, (r0 + hh) * 512:(r0 + hh + 1) * 512],
                                         start=True, stop=True,
                                         skip_group_check=True)
                    rb = rbp.tile([64, 1024], F16, tag="rb16")
                    nc.scalar.activation(rb[:, 0:512 * nr], ps[0:64, 0:512 * nr],
                                         PRELU, bias=bt[0][:, 0:1], scale=1.0,
                                         alpha=at[0][:, 0:1])
                    nc.sync.dma_start(yb[0][:, c0 + r0: c0 + r0 + nr, :],
                                      rb[:, 0:512 * nr].rearrange(
                                          "c (r w) -> c r w", w=512))

            # ---------------- phase C: layers 1-3 ----------------
            for l in (1, 2, 3):
                wyt = wp.tile([128, 960], F16, tag=f"wy{l}")
                nc.sync.dma_start(wyt[:], wy_e[l][:, :])
                wxt = wp.tile([125, 64], F16, tag=f"wx{l}")
                nc.sync.dma_start(wxt[:], wx_e[l][:, :])
                ybin = yb[l - 1]
                reg_in, reg_out = REGY[l - 1], REGY[l]
                dst = out_e if l == 3 else yb[l]
                for c0, rc in _chunks(reg_out, rchunk):
                    nrows_t = rc + 4
                    yt = yp.tile([128, (rchunk + 4) * PITCH], F16, tag="yt")
                    ytv = yt[:].rearrange("c (r p) -> c r p", p=PITCH)
                    # main load rows [c0-2, c0+rc+2) of ybin, reflected at 0
                    if c0 == 0:
                        nc.sync.dma_start(ytv[0:64, 2:nrows_t, 2:514],
                                          ybin[:, 0:rc + 2, :])
                        for j, srow in ((0, 2), (1, 1)):
                            nc.sync.dma_start(ytv[0:64, j, 2:514],
                                              ybin[:, srow, :])
                            for d, s in COLPAD:
                                nc.sync.dma_start(ytv[0:64, j, d:d + 1],
                                                  ybin[:, srow, s:s + 1])
                        for d, s in COLPAD:
                            nc.sync.dma_start(ytv[0:64, 2:nrows_t, d:d + 1],
                                              ybin[:, 0:rc + 2, s:s + 1])
                    else:
                        nc.sync.dma_start(ytv[0:64, 0:nrows_t, 2:514],
                                          ybin[:, c0 - 2:c0 + rc + 2, :])
                        for d, s in COLPAD:
                            nc.sync.dma_start(ytv[0:64, 0:nrows_t, d:d + 1],
                                              ybin[:, c0 - 2:c0 + rc + 2, s:s + 1])
                    # dup: upper partitions = lower shifted one row
                    if "dup" not in skips:
                        nc.sync.dma_start(yt[64:128, 0:(nrows_t - 1) * PITCH],
                                          yt[0:64, PITCH:nrows_t * PITCH])
                    xs = build_xst(c0, rc)
                    for r0 in range(0, rc, 2):
                        nr = min(2, rc - r0)
                        ps = pscp.tile([128, 1024], F32, tag="ps")
                        for hh in range(nr):
                            rr = r0 + hh
                            ocs = slice(512 * hh, 512 * (hh + 1))
                            # colA: pair0 dx0-4 (K=128), singles dx0-2 (K=64)
                            mmsA = [(wyt[:, (0 * 5 + dx) * 64:(0 * 5 + dx) * 64 + 64],
                                     yt[:, (rr + 0) * PITCH + dx:(rr + 0) * PITCH + dx + 512])
                                    for dx in range(5)]
                            mmsA += [(wyt[0:64, (10 + dx) * 64:(10 + dx) * 64 + 64],
                                      yt[0:64, (rr + 4) * PITCH + dx:(rr + 4) * PITCH + dx + 512])
                                     for dx in range(3)]
                            # colB: pair1 dx0-4 (K=128), singles dx3-4, x
                            mmsB = [(wyt[:, (1 * 5 + dx) * 64:(1 * 5 + dx) * 64 + 64],
                                     yt[:, (rr + 2) * PITCH + dx:(rr + 2) * PITCH + dx + 512])
                                    for dx in range(5)]
                            mmsB += [(wyt[0:64, (10 + dx) * 64:(10 + dx) * 64 + 64],
                                      yt[0:64, (rr + 4) * PITCH + dx:(rr + 4) * PITCH + dx + 512])
                                     for dx in (3, 4)]
                            if "xmm" in skips:
                                mmsB += [(wxt[:], xs[:, 0:512])]
                            else:
                                mmsB += [(wxt[:], xs[:, rr * 512:(rr + 1) * 512])]
                            for k in range(8):
                                la, ra = mmsA[k]
                                nc.tensor.matmul(ps[0:64, ocs], la, ra,
                                                 start=(k == 0), stop=(k == 7),
                                                 skip_group_check=True)
                                lb, rb_ = mmsB[k]
                                nc.tensor.matmul(ps[64:128, ocs], lb, rb_,
                                                 start=(k == 0), stop=(k == 7),
                                                 skip_group_check=True,
                                                 tile_position=(0, 64))
                        w1024 = 512 * nr
                        ct = cp.tile([64, 1024], F32, tag="ct")
                        nc.scalar.copy(ct[:, 0:w1024], ps[64:128, 0:w1024])
                        tt = cp.tile([64, 1024], F32, tag="tt")
                        nc.vector.tensor_add(tt[:, 0:w1024], ps[0:64, 0:w1024],
                                             ct[:, 0:w1024])
                        if l == 3:
                            rb = rbp.tile([64, 1024], F32, tag="rb32")
                        else:
                            rb = rbp.tile([64, 1024], F16, tag="rb16")
                        nc.scalar.activation(rb[:, 0:w1024], tt[:, 0:w1024],
                                             PRELU, bias=bt[l][:, 0:1], scale=1.0,
                                             alpha=at[l][:, 0:1])
                        nc.sync.dma_start(
                            dst[:, c0 + r0: c0 + r0 + nr, :],
                            rb[:, 0:w1024].rearrange("c (r w) -> c r w", w=512))
    nc.compile()
    return nc


def _get_exec(nrows, rchunk):
    key = (nrows, rchunk)
    if key in _CACHE:
        return _CACHE[key]
    import jax
    import concourse.mybir as mybir
    from jax.sharding import Mesh, PartitionSpec
    from jax.experimental.shard_map import shard_map
    from concourse import bass2jax
    from concourse.bass2jax import _bass_exec_p, install_neuronx_cc_hook

    nc = _build_program(nrows, rchunk)
    install_neuronx_cc_hook()

    part_name = nc.partition_id_tensor.name if nc.partition_id_tensor else None
    in_names, out_names, out_avals, zero_shapes = [], [], [], []
    for alloc in nc.m.functions[0].allocations:
        if not isinstance(alloc, mybir.MemoryLocationSet):
            continue
        name = alloc.memorylocations[0].name
        if alloc.kind == "ExternalInput":
            if name != part_name:
                in_names.append(name)
        elif alloc.kind == "ExternalOutput":
            shape = tuple(alloc.tensor_shape)
            dtype = mybir.dt.np(alloc.dtype)
            out_names.append(name)
            out_avals.append(jax.core.ShapedArray(shape, dtype))
            zero_shapes.append((shape, dtype))
    n_params = len(in_names)
    n_outs = len(out_names)
    all_names = in_names + out_names
    if part_name is not None:
        all_names = all_names + [part_name]

    import jax.numpy as jnp

    def _call_once(ins, out_bufs):
        operands = list(ins) + list(out_bufs)
        if part_name is not None:
            operands.append(bass2jax.partition_id_tensor())
        outs = _bass_exec_p.bind(
            *operands,
            out_avals=tuple(out_avals),
            in_names=tuple(all_names),
            out_names=tuple(out_names),
            lowering_input_output_aliases=(),
            sim_require_finite=True,
            sim_require_nnan=True,
            nc=nc,
        )
        return tuple(outs)

    def _body_iters(iters):
        def f(*args):
            ins = args[:n_params]
            bufs = list(args[n_params:n_params + n_outs])
            for _ in range(iters):
                bufs = list(_call_once(ins, bufs))
            return tuple(bufs)
        return f

    _body = _body_iters(1)

    devices = jax.devices()[:8]
    mesh = Mesh(np.asarray(devices), ("core",))
    in_specs = (PartitionSpec("core"),) * (n_params + n_outs)
    out_specs = (PartitionSpec("core"),) * n_outs
    donate = tuple(range(n_params, n_params + n_outs))
    sharded = jax.jit(
        shard_map(_body, mesh=mesh, in_specs=in_specs, out_specs=out_specs,
                  check_rep=False),
        donate_argnums=donate, keep_unused=True)

    def _concat_in(in_maps):
        return [np.concatenate([np.asarray(m[name]) for m in in_maps], axis=0)
                for name in in_names]

    def _concat_zeros():
        return [np.zeros((8 * s[0], *s[1:]), d) for s, d in zero_shapes]

    def run(in_maps):
        out_arrs = sharded(*_concat_in(in_maps), *_concat_zeros())
        return [
            {name: np.asarray(out_arrs[i]).reshape(8, *out_avals[i].shape)[c]
             for i, name in enumerate(out_names)}
            for c in range(8)
        ]

    def make_timer(in_maps):
        """Returns sample() -> wall seconds of one execution, device I/O."""
        import time as _time
        dev_in = [jax.device_put(x) for x in _concat_in(in_maps)]
        fn = jax.jit(
            shard_map(_body, mesh=mesh, in_specs=in_specs,
                      out_specs=out_specs, check_rep=False),
            keep_unused=True)
        zz = [jax.device_put(z) for z in _concat_zeros()]
        outs = fn(*dev_in, *zz)          # compile + warm
        jax.block_until_ready(outs)

        def sample():
            t0 = _time.time()
            o = fn(*dev_in, *zz)
            jax.block_until_ready(o)
            return _time.time() - t0
        return sample

    def time_exec(in_maps, repeats=10):
        s = make_timer(in_maps)
        return min(s() for _ in range(repeats))

    run.time_exec = time_exec
    run.make_timer = make_timer
    _CACHE[key] = run
    return run


def baseline_time(repeats=10):
    """Time an (almost) empty program with the same output signature, to
    subtract dispatch/RPC overhead from time_exec."""
    if "baseline" in _CACHE:
        return _CACHE["baseline"](repeats)
    import jax
    import concourse.tile as tile
    from concourse import bacc, mybir

    F32 = mybir.dt.float32
    nc = bacc.Bacc("TRN2", target_bir_lowering=False, debug=False, num_devices=8)
    x_e = nc.dram_tensor("x", [64, 512], F32, kind="ExternalInput")
    out_e = nc.dram_tensor("out", [64, 256, 512], F32, kind="ExternalOutput")
    with tile.TileContext(nc) as tc:
        with tc.tile_pool(name="sb", bufs=1) as sb:
            t = sb.tile([64, 512], F32)
            nc.sync.dma_start(t[:], x_e[:, :])
            nc.sync.dma_start(out_e[:, 0, :], t[:])
    nc.compile()
    runner = _wrap_exec(nc)

    import numpy as _np
    in_maps = [{"x": _np.zeros((64, 512), _np.float32)} for _ in range(8)]
    sampler = runner(in_maps)

    def bt(reps):
        return min(sampler() for _ in range(reps))

    bt.sample = sampler
    _CACHE["baseline"] = bt
    return bt(repeats)


def _wrap_exec(nc):
    """Minimal timed executor for an arbitrary compiled nc (used by baseline)."""
    import jax
    import concourse.mybir as mybir
    from jax.sharding import Mesh, PartitionSpec
    from jax.experimental.shard_map import shard_map
    from concourse import bass2jax
    from concourse.bass2jax import _bass_exec_p, install_neuronx_cc_hook
    install_neuronx_cc_hook()

    part_name = nc.partition_id_tensor.name if nc.partition_id_tensor else None
    in_names, out_names, out_avals, zero_shapes = [], [], [], []
    for alloc in nc.m.functions[0].allocations:
        if not isinstance(alloc, mybir.MemoryLocationSet):
            continue
        name = alloc.memorylocations[0].name
        if alloc.kind == "ExternalInput":
            if name != part_name:
                in_names.append(name)
        elif alloc.kind == "ExternalOutput":
            shape = tuple(alloc.tensor_shape)
            dtype = mybir.dt.np(alloc.dtype)
            out_names.append(name)
            out_avals.append(jax.core.ShapedArray(shape, dtype))
            zero_shapes.append((shape, dtype))
    n_params, n_outs = len(in_names), len(out_names)
    all_names = in_names + out_names + ([part_name] if part_name else [])

    def _body(*args):
        operands = list(args)
        if part_name is not None:
            operands.append(bass2jax.partition_id_tensor())
        return tuple(_bass_exec_p.bind(
            *operands, out_avals=tuple(out_avals), in_names=tuple(all_names),
            out_names=tuple(out_names), lowering_input_output_aliases=(),
            sim_require_finite=True, sim_require_nnan=True, nc=nc))

    devices = jax.devices()[:8]
    mesh = Mesh(np.asarray(devices), ("core",))
    fn = jax.jit(
        shard_map(_body, mesh=mesh,
                  in_specs=(PartitionSpec("core"),) * (n_params + n_outs),
                  out_specs=(PartitionSpec("core"),) * n_outs,
                  check_rep=False),
        keep_unused=True)

    def timed(in_maps):
        import time as _time
        dev_in = [jax.device_put(
            np.concatenate([np.asarray(m[nm]) for m in in_maps], axis=0))
            for nm in in_names]
        zz = [jax.device_put(np.zeros((8 * s[0], *s[1:]), d))
              for s, d in zero_shapes]
        outs = fn(*dev_in, *zz)
        jax.block_until_ready(outs)

        def sample():
            t0 = _time.time()
            o = fn(*dev_in, *zz)
            jax.block_until_ready(o)
            return _time.time() - t0
        return sample

    return timed


def _make_in_maps(inputs, nrows):
    nx = np.asarray(inputs["nx"], np.float32)        # [4, 1, 512, 512]
    gks = [np.asarray(inputs[f"gk{i}"], np.float32) for i in range(4)]
    gfs, gwe, gwo = _build_g_matrices(gks, nrows)
    packs_even = [_pack_weights(inputs[f"w{l}"], False) for l in range(4)]
    packs_odd = [_pack_weights(inputs[f"w{l}"], True) for l in range(4)]
    in_maps = []
    for c in range(8):
        s, half = c >> 1, c & 1
        img = nx[s, 0]
        if half:
            img = img[::-1, :]
        m = {"nx16": np.ascontiguousarray(img).astype(np.float16)}
        gw = gwo if half else gwe
        for i in range(4):
            m[f"gw{i}"] = gw[i]
            m[f"gf{i}"] = gfs[i]
        packs = packs_odd if half else packs_even
        m["w0s"] = packs[0][1]
        for l in (1, 2, 3):
            m[f"wy{l}"] = packs[l][0]
            m[f"wx{l}"] = packs[l][1]
        for l in range(4):
            m[f"b{l}"] = np.asarray(inputs[f"b{l}"], np.float32)
            m[f"a{l}"] = np.asarray(inputs[f"a{l}"], np.float32).reshape(1)
        in_maps.append(m)
    return in_maps


def kernel(**inputs) -> np.ndarray:
    nrows = int(os.environ.get("BK_NROWS", HALF))
    rchunk = int(os.environ.get("BK_RCHUNK", 32))
    run = _get_exec(nrows, rchunk)
    in_maps = _make_in_maps(inputs, nrows)
    results = run(in_maps)
    out = np.zeros((B, 64, H, W), np.float32)
    for c in range(8):
        s, half = c >> 1, c & 1
        o = results[c]["out"]                      # [64, nrows, 512]
        if half:
            out[s, :, H - nrows:H, :] = o[:, ::-1, :]
        else:
            out[s, :, 0:nrows, :] = o
    return out
